# revision 81
# baseline (speedup 1.0000x reference)
"""Trainium2 Bass kernel for nn_FFTResonanceLookup.

Full inputs: selections (8,16,4,512) f32, items (512,771) f32.
Output: (8,16,4,32768) f32, unit-normalized along samples.

Data-parallel over the 512 (b,e,x) rows -> 64 rows/core x 8 cores.
Per row the synth is a 1024-feature x 256-sample matmul (irfft + hann
window + 50%-overlap-add folded into constant weights). Spectra come
from exp/sin activations; phase range reduction is a DVE floor-mod
with the half-turn shift folded into the arg matmul so that
sin(2*pi*u - pi) returns exact sin/cos. Normalization is done on host
(device returns unscaled rows; host also adds the constant t=0-slot
correction before normalizing).
"""

import math
from contextlib import ExitStack

import numpy as np

N_ITEMS = 512
N_COEFFS = 771
CBINS = 257
WIN = 512
STEP = 256
NFR = 128
NT = NFR + 1            # t = 0..128
NT_S = 136              # padded per-row stride: 64*136 = 8704 = 17*512
R_PER_CORE = 64
N_CORES = 8
NROWS = 512
PIECE = 512
NPIECE = 17
FREE = R_PER_CORE * NT_S  # 8704

# piece p -> rows whose DFT windows are fully covered once piece p done
ROWS_AT_PIECE = [[] for _ in range(NPIECE)]
for _r in range(R_PER_CORE):
    ROWS_AT_PIECE[(_r * NT_S + NFR) // PIECE].append(_r)

ASL_CHUNKS = 8          # ampsel DMA split for early pipeline start
# staging-copy engine cycle: g=gpsimd(Pool), v=vector(DVE), a=scalar(Act)
COPY_CYCLE = "a"
OUT_GROUP = 4           # rows per output DMA

_BUILT = None


def _build_consts():
    import ml_dtypes
    hann = np.hanning(WIN)
    k = np.arange(CBINS)[:, None]
    n = np.arange(WIN)[None, :]
    ang = 2.0 * np.pi * k * n / WIN
    Cm = np.cos(ang) / WIN * np.where((k >= 1) & (k <= 255), 2.0, 1.0)
    Sm = -np.sin(ang) / WIN * np.where((k >= 1) & (k <= 255), 2.0, 0.0)
    Cw = Cm * hann[None, :]
    Sw = Sm * hann[None, :]
    W1 = np.concatenate([Cw[:256, :STEP], Sw[:256, :STEP]], 0)   # (512,256)
    W2 = np.concatenate([Cw[:256, STEP:], Sw[:256, STEP:]], 0)   # (512,256)
    wm = np.zeros((128, 8 * 256))
    for j in range(4):
        wm[:, 256 * j:256 * (j + 1)] = W1[128 * j:128 * (j + 1), :]
        wm[:, 256 * (j + 4):256 * (j + 5)] = W2[128 * j:128 * (j + 1), :]
    # bin 256 is added on the host (rank-1 per row); the im-bin-0 rows of
    # the Sw chunks stay zero, so IM[0] partition 0 can hold anything.
    wm_bf = wm.astype(ml_dtypes.bfloat16)
    w256 = Cw[256, :]                                # (512,) f64 basis

    # every row's t=0 slot produces RE = -1 (amp=1, cos arg = -pi) across all
    # 256 bins; its W2 contribution (chunks 4,5) is a constant per-sample
    # vector cancelled on host after the device run.
    wm_f64 = wm_bf.astype(np.float64)
    corr = (wm_f64[:, 4 * 256:5 * 256].sum(0)
            + wm_f64[:, 5 * 256:6 * 256].sum(0)).astype(np.float32)

    t = np.arange(NT, dtype=np.float64)
    sel_t = np.zeros((64, FREE))
    sel_1 = np.zeros((64, FREE))
    for r in range(64):
        sel_t[r, r * NT_S:r * NT_S + NT] = t
        sel_1[r, r * NT_S + 1:r * NT_S + NT] = 1.0   # t=0 slot stays 0
    ampsel = np.concatenate([sel_t, sel_1], 0)       # (128,8704)
    return (wm_bf, ampsel.astype(np.float32), corr, w256)


def _kernel_body(ctx: ExitStack, tc, out_d, sel_d, items_d, wm_d,
                 ampsel_d, ident_d, negi_d):
    import concourse.mybir as mybir

    nc = tc.nc
    f32 = mybir.dt.float32
    f32r = mybir.dt.float32r
    f16 = mybir.dt.float16
    bf16 = mybir.dt.bfloat16
    AF = mybir.ActivationFunctionType
    OP = mybir.AluOpType
    PI = math.pi
    MAGIC = 12582912.0

    const = ctx.enter_context(tc.tile_pool(name="const", bufs=1))
    persist = ctx.enter_context(tc.tile_pool(name="persist", bufs=1))

    _pn = [0]

    def p2tile(shape, name):
        _pn[0] += 1
        return pst2.tile(shape, f32, tag="p2", name=f"{name}{_pn[0]}")

    # scoped pools: ampsel (dead after phase 1) and stage-1/2 transients
    # (dead after stage 2); released zones are reused by phase-2 pools.
    # Created in reverse release order (pool stack is LIFO).
    ppam = tc.tile_pool(name="ppam", bufs=2, space="PSUM")
    ppa = ppam.__enter__()
    pst2p = tc.tile_pool(name="pst2", bufs=2, space="PSUM")
    pst2 = pst2p.__enter__()
    s2p = tc.tile_pool(name="s2p", bufs=1)
    s2 = s2p.__enter__()

    # ---- constants + inputs (SP DMA queue is in-order: stage-2 inputs
    # first, then ampsel chunks, wm last — first needed only at first DFT)
    it4 = s2.tile([128, 4 * N_COEFFS + 4], f32)
    items_v = items_d.rearrange("(a p) c -> p a c", p=128)
    nc.sync.dma_start(it4[:, 0:4 * N_COEFFS].rearrange("p (a c) -> p a c", a=4),
                      items_v)
    zpad = s2.tile([128, 4], f32)
    nc.vector.memset(zpad[:], 0.0)
    nc.vector.tensor_copy(it4[:, 4 * N_COEFFS:], zpad[:])
    sel_t = s2.tile([64, 512], f32)
    nc.sync.dma_start(sel_t[:], sel_d[:])
    ident = const.tile([128, 128], f32)
    nc.sync.dma_start(ident[:], ident_d[:])
    asl = const.tile([128, FREE], f32r)
    ck = FREE // ASL_CHUNKS
    for i in range(ASL_CHUNKS):
        nc.sync.dma_start(asl[:, i * ck:(i + 1) * ck],
                          ampsel_d[:, i * ck:(i + 1) * ck])
    wm = const.tile([128, 2048], bf16)
    nc.sync.dma_start(wm[:], wm_d[:])
    bias_half = const.tile([128, 1], f32)
    nc.vector.memset(bias_half[:], 0.5)
    bias_npi = const.tile([128, 1], f32)
    nc.vector.memset(bias_npi[:], -PI)
    zeros64 = const.tile([64, 128], f32)
    nc.vector.memset(zeros64[:], 0.0)
    quart64 = const.tile([64, 128], f32)
    nc.vector.memset(quart64[:], 0.25)
    negi = const.tile([128, 128], f16)
    nc.sync.dma_start(negi[:], negi_d[:])

    rs = s2.tile([64, 512], f32)
    nc.scalar.activation(rs[:], sel_t[:], AF.Relu)

    rsT = []
    for kc in range(4):
        pt_ = p2tile([128, 64], "tr")
        nc.tensor.transpose(pt_[:], rs[:, kc * 128:(kc + 1) * 128],
                            ident[0:64, 0:64])
        st = s2.tile([128, 64], f32, name=f"rsT{kc}")
        nc.vector.tensor_copy(st[:], pt_[:])
        rsT.append(st)

    coefA = s2.tile([64, N_COEFFS], f32)
    pA1 = p2tile([64, 512], "pA")
    for kc in range(4):
        nc.tensor.matmul(pA1[:], rsT[kc][:], it4[:, kc * N_COEFFS:kc * N_COEFFS + 512],
                         start=(kc == 0), stop=(kc == 3))
    pA2 = p2tile([64, 260], "pA")
    for kc in range(4):
        # 260-wide (f32r needs even free size); col 259 is padding/garbage
        nc.tensor.matmul(pA2[:], rsT[kc][:],
                         it4[:, kc * N_COEFFS + 512:kc * N_COEFFS + 772],
                         start=(kc == 0), stop=(kc == 3))
    nc.vector.tensor_copy(coefA[:, 0:512], pA1[:])
    nc.vector.tensor_copy(coefA[:, 512:771], pA2[:, 0:259])

    # ---- stage 2: activation blocks ----
    # block A: sigmoid table (sigmoid + tanh)
    sig_mag, sig_st, th = {}, {}, {}
    for c in range(2):
        sm = s2.tile([64, 128], f32, name=f"sigmag{c}")
        nc.scalar.activation(sm[:], coefA[:, c * 128:(c + 1) * 128], AF.Sigmoid)
        sig_mag[c] = sm
        ss = s2.tile([64, 128], f32, name=f"sigst{c}")
        nc.scalar.activation(ss[:], coefA[:, 2 * CBINS + c * 128:2 * CBINS + (c + 1) * 128],
                             AF.Sigmoid)
        sig_st[c] = ss
        tt = s2.tile([64, 128], f32, name=f"th{c}")
        nc.scalar.activation(tt[:], coefA[:, CBINS + c * 128:CBINS + (c + 1) * 128],
                             AF.Tanh)
        th[c] = tt

    # weight tiles for the arg matmuls
    ampx, thT, thTc = {}, {}, {}
    for c in range(2):
        ax = persist.tile([128, 128], f32r, name=f"ampx{c}")
        ampx[c] = ax
        tx = persist.tile([128, 128], f32r, name=f"thT{c}")
        nc.vector.tensor_copy(tx[64:128, :], zeros64[:])
        nc.vector.tensor_scalar(tx[0:64, :], th[c][:], 0.5, None, OP.mult)
        thT[c] = tx
        txc = persist.tile([128, 128], f32r, name=f"thTc{c}")
        nc.vector.tensor_copy(txc[64:128, :], quart64[:])
        nc.vector.tensor_copy(txc[0:64, :], tx[0:64, :])
        thTc[c] = txc

    # block B: natural_log_exp table (Ln now, Exp pieces later share it).
    # The fence keeps the scheduler from interleaving sigmoid/tanh with Ln,
    # which would thrash activation tables.
    tc.no_sync_barrier()
    for c in range(2):
        lgm = s2.tile([64, 128], f32, name=f"lgm{c}")
        nc.scalar.activation(lgm[:], sig_mag[c][:], AF.Ln,
                             bias=bias_half[0:64], scale=0.49995)
        nc.vector.tensor_copy(ampx[c][0:64, :], lgm[:])
        lgs = s2.tile([64, 128], f32, name=f"lgs{c}")
        nc.scalar.activation(lgs[:], sig_st[c][:], AF.Ln)
        nc.vector.tensor_copy(ampx[c][64:128, :], lgs[:])
    s2p.__exit__(None, None, None)
    pst2p.__exit__(None, None, None)

    # ---- stage 3+4: two phases (exp+mods, then sin+DFT) ----
    AMP = [persist.tile([128, FREE], bf16, name=f"AMP{i}") for i in range(2)]

    specs = [None, None, None, None]
    cp_i = [0]
    ost_state = {"tile": None, "po": None}
    pending = []            # completed po pair tiles awaiting staging copy

    def emit_row_mm(r):
        # two rows share one PSUM bank; the pair joins `pending` on the
        # odd row and is staged a couple of pieces later (so the staging
        # copy never heads-of-line-blocks the DVE queue behind PE)
        half = r % 2
        if half == 0:
            _pn[0] += 1
            ost_state["po"] = ppo.tile([128, 512], f32, tag="po",
                                       name=f"po{_pn[0]}")
        po = ost_state["po"]
        pv = po[:, half * 256:(half + 1) * 256]
        for j in range(4):
            nc.tensor.matmul(pv,
                             specs[j][:, r * NT_S + 1:r * NT_S + NT],
                             wm[:, j * 256:(j + 1) * 256],
                             start=(j == 0), stop=False)
        for j in range(4):
            nc.tensor.matmul(pv,
                             specs[j][:, r * NT_S:r * NT_S + NFR],
                             wm[:, (j + 4) * 256:(j + 5) * 256],
                             start=False, stop=(j == 3))
        if half == 1:
            pending.append((r - 1, po))

    def flush_pairs(n=None):
        k = len(pending) if n is None else min(n, len(pending))
        for _ in range(k):
            r0p, po = pending.pop(0)
            slot = r0p % OUT_GROUP
            if slot == 0:
                ost_state["tile"] = ostage.tile(
                    [128, OUT_GROUP * 256], f32, tag="ost",
                    name=f"ost{r0p // OUT_GROUP}")
            ost = ost_state["tile"]
            dst = ost[:, slot * 256:(slot + 2) * 256]
            eng = COPY_CYCLE[cp_i[0] % len(COPY_CYCLE)]
            cp_i[0] += 1
            if eng == "v":
                nc.vector.tensor_copy(dst, po[:])
            else:
                nc.scalar.copy(dst, po[:])
            if slot == OUT_GROUP - 2:
                r0 = r0p - OUT_GROUP + 2
                dstv = out_d[r0:r0 + OUT_GROUP, :].rearrange(
                    "r (f s) -> f r s", f=NFR)
                nc.sync.dma_start(dstv, ost[:].rearrange(
                    "f (r s) -> f r s", r=OUT_GROUP))

    # --- phase 1: arg matmuls + Exp activations (Ln/Exp share a table) ---
    tc.no_sync_barrier()
    for p in range(NPIECE):
        w0, w1 = p * PIECE, (p + 1) * PIECE
        for c in range(2):
            _pn[0] += 1
            pa = ppa.tile([128, PIECE], f32, tag="pa", name=f"pa{_pn[0]}")
            nc.tensor.matmul(pa[:], ampx[c][:], asl[:, w0:w1],
                             start=True, stop=True)
            nc.scalar.activation(AMP[c][:, w0:w1], pa[:], AF.Exp)
    ppam.__exit__(None, None, None)

    # --- phase 2: theta matmul pairs, magic-round + negi subtract (u stays
    # f32 in PSUM), one Sin per unit, spec products, DFT + output.
    pptp = ctx.enter_context(tc.tile_pool(name="pptp", bufs=2, space="PSUM"))
    ppo = ctx.enter_context(tc.tile_pool(name="ppo", bufs=4, space="PSUM"))
    reim = ctx.enter_context(tc.tile_pool(name="reim", bufs=1))
    rspool = ctx.enter_context(tc.tile_pool(name="rspool", bufs=3))
    scpool = ctx.enter_context(tc.tile_pool(name="scpool", bufs=3))
    ostage = ctx.enter_context(tc.tile_pool(name="ostage", bufs=2))
    RE = [reim.tile([128, FREE], bf16, name=f"RE{i}") for i in range(2)]
    IM = [reim.tile([128, FREE], bf16, name=f"IM{i}") for i in range(2)]
    specs[:] = [RE[0], RE[1], IM[0], IM[1]]

    units = [(p, c) for p in range(NPIECE) for c in range(2)]
    ustate = {}

    def emit_ptmm_rs(i):
        p, c = units[i]
        w0, w1 = p * PIECE, (p + 1) * PIECE
        _pn[0] += 1
        ptp = pptp.tile([128, 2 * PIECE], f32, tag="ptp", name=f"ptp{_pn[0]}")
        nc.tensor.matmul(ptp[:, 0:PIECE], thT[c][:], asl[:, w0:w1],
                         start=True, stop=True)
        nc.tensor.matmul(ptp[:, PIECE:], thTc[c][:], asl[:, w0:w1],
                         start=True, stop=True)
        rs_ = rspool.tile([128, 2 * PIECE], f16, tag="rs", name=f"rs{_pn[0]}")
        nc.vector.tensor_scalar(rs_[:], ptp[:], MAGIC, MAGIC,
                                OP.add, OP.subtract)
        ustate[i] = (ptp, rs_)

    def emit_unit_tail(i):
        p, c = units[i]
        w0, w1 = p * PIECE, (p + 1) * PIECE
        ptp, rs_ = ustate.pop(i)
        nc.tensor.matmul(ptp[:, 0:PIECE], negi[:], rs_[:, 0:PIECE],
                         start=False, stop=True, skip_group_check=True)
        nc.tensor.matmul(ptp[:, PIECE:], negi[:], rs_[:, PIECE:],
                         start=False, stop=True, skip_group_check=True)
        _pn[0] += 1
        sc = scpool.tile([128, 2 * PIECE], bf16, tag="sc", name=f"sc{_pn[0]}")
        nc.scalar.activation(sc[:], ptp[:], AF.Sin, scale=2.0 * PI)
        # sc[:, :PIECE] = sin, sc[:, PIECE:] = cos
        nc.vector.tensor_tensor(RE[c][:, w0:w1], AMP[c][:, w0:w1],
                                sc[:, PIECE:], OP.mult)
        nc.vector.tensor_tensor(IM[c][:, w0:w1], AMP[c][:, w0:w1],
                                sc[:, 0:PIECE], OP.mult)

    tc.no_sync_barrier()
    emit_ptmm_rs(0)
    for i in range(len(units)):
        p, c = units[i]
        if i + 1 < len(units):
            emit_ptmm_rs(i + 1)
        emit_unit_tail(i)
        if c == 1:
            if len(pending) > 1:
                flush_pairs(len(pending) - 1)
            for r in ROWS_AT_PIECE[p]:
                emit_row_mm(r)
    flush_pairs()


def _build():
    global _BUILT
    if _BUILT is not None:
        return _BUILT
    import concourse.bacc as bacc
    import concourse.mybir as mybir
    import concourse.tile as tile

    wm_np, ampsel_np, corr_np, w256_np = _build_consts()
    ident_np = np.eye(128, dtype=np.float32)
    negi_np = -np.eye(128, dtype=np.float16)

    nc = bacc.Bacc("TRN2", target_bir_lowering=False, debug=False,
                   num_devices=N_CORES)
    f32 = mybir.dt.float32
    bf16 = mybir.dt.bfloat16
    sel_d = nc.dram_tensor("sel", [R_PER_CORE, N_ITEMS], f32,
                           kind="ExternalInput").ap()
    items_d = nc.dram_tensor("items", [N_ITEMS, N_COEFFS],
                             f32, kind="ExternalInput").ap()
    wm_d = nc.dram_tensor("wm", list(wm_np.shape), bf16,
                          kind="ExternalInput").ap()
    ampsel_d = nc.dram_tensor("ampsel", list(ampsel_np.shape),
                              mybir.dt.float32r, kind="ExternalInput").ap()
    ident_d = nc.dram_tensor("ident", [128, 128], f32,
                             kind="ExternalInput").ap()
    negi_d = nc.dram_tensor("negi", [128, 128], mybir.dt.float16,
                            kind="ExternalInput").ap()
    out_d = nc.dram_tensor("out", [R_PER_CORE, NFR * STEP], f32,
                           kind="ExternalOutput").ap()

    with tile.TileContext(nc) as tc:
        with ExitStack() as ctx:
            _kernel_body(ctx, tc, out_d, sel_d, items_d, wm_d,
                         ampsel_d, ident_d, negi_d)
    nc.compile()

    _BUILT = (nc, wm_np, ampsel_np, ident_np, negi_np, corr_np, w256_np)
    return _BUILT


def _in_maps(selections, items, wm_np, ampsel_np, ident_np, negi_np):
    sel_flat = np.ascontiguousarray(
        np.asarray(selections).reshape(NROWS, N_ITEMS).astype(np.float32))
    items_f = np.ascontiguousarray(np.asarray(items).astype(np.float32))
    maps = []
    for c in range(N_CORES):
        maps.append({
            "sel": sel_flat[c * R_PER_CORE:(c + 1) * R_PER_CORE],
            "items": items_f,
            "wm": wm_np,
            "ampsel": ampsel_np,
            "ident": ident_np,
            "negi": negi_np,
        })
    return maps


def _bin256_host(selections, items, w256):
    # bin 256 (Nyquist) spectral line, computed exactly on host: per row a
    # rank-1 [frames x window] contribution folded with hann + overlap-add
    sel = np.maximum(np.asarray(selections, np.float64).reshape(NROWS, N_ITEMS),
                     0.0)
    it = np.asarray(items, np.float64)[:, [256, 256 + CBINS, 256 + 2 * CBINS]]
    c3 = sel @ it                                            # (512, 3)
    mag = 0.5 + 1.0 / (1.0 + np.exp(-c3[:, 0])) * 0.49995
    ph = np.tanh(c3[:, 1]) * np.pi
    st = 1.0 / (1.0 + np.exp(-c3[:, 2]))
    t = np.arange(NT, dtype=np.float64)                      # 0..128
    spec = (st[:, None] * np.exp(np.log(mag + 1e-12)[:, None] * t)
            * np.cos(ph[:, None] * t))                       # (512, 129)
    sw1 = spec[:, 1:NT]                                      # t = f+1
    sw2 = spec[:, 0:NFR].copy()
    sw2[:, 0] = 0.0                                          # no frame -1
    return (sw1[:, :, None] * w256[None, None, :STEP]
            + sw2[:, :, None] * w256[None, None, STEP:])     # (512,128,256)


def kernel(selections: np.ndarray, items: np.ndarray) -> np.ndarray:
    from concourse.bass_utils import run_bass_kernel_spmd

    nc, wm_np, ampsel_np, ident_np, negi_np, corr_np, w256_np = _build()
    in_maps = _in_maps(selections, items, wm_np, ampsel_np, ident_np, negi_np)
    res = run_bass_kernel_spmd(nc, in_maps, core_ids=list(range(N_CORES)))
    rows = np.concatenate([res.results[c]["out"] for c in range(N_CORES)], 0)
    rows = rows.astype(np.float64)
    # t=0 slot: theta-const rides sel_1 (zeroed at t0), so cos(t0)=sin(0)=0
    # and RE(t0)=0 on device already -- no correction needed
    rows += _bin256_host(selections, items, w256_np).reshape(NROWS, -1)
    norms = np.linalg.norm(rows, axis=-1, keepdims=True)
    rows = rows / (norms + 1e-8)
    sh = np.asarray(selections).shape
    return rows.reshape(sh[0], sh[1], sh[2], NFR * STEP).astype(np.float32)


# revision 90
# speedup vs baseline: 1.0465x; 1.0465x over previous
"""Trainium2 Bass kernel for nn_FFTResonanceLookup.

Full inputs: selections (8,16,4,512) f32, items (512,771) f32.
Output: (8,16,4,32768) f32, unit-normalized along samples.

Data-parallel over the 512 (b,e,x) rows -> 64 rows/core x 8 cores.
Per row the synth is a 1024-feature x 256-sample matmul (irfft + hann
window + 50%-overlap-add folded into constant weights). Two phases so
the Act engine never thrashes activation tables: (1) all arg matmuls +
Exp (Ln/Exp share a table), (2) theta matmul pairs with magic-number
round + negi-matmul subtract (reduced phase stays f32 in PSUM), one
Sin per unit, spec products, and per-row DFT matmuls interleaved as
spec pieces complete. Bin 256 (Nyquist) is a rank-1 per-row term added
on host in f64; normalization is also on host.
"""

import math
from contextlib import ExitStack

import numpy as np

N_ITEMS = 512
N_COEFFS = 771
CBINS = 257
WIN = 512
STEP = 256
NFR = 128
NT = NFR + 1            # t = 0..128
NT_S = 136              # padded per-row stride: 64*136 = 8704 = 17*512
R_PER_CORE = 64
N_CORES = 8
NROWS = 512
PIECE = 512
NPIECE = 17
FREE = R_PER_CORE * NT_S  # 8704

# piece p -> rows whose DFT windows are fully covered once piece p done
ROWS_AT_PIECE = [[] for _ in range(NPIECE)]
for _r in range(R_PER_CORE):
    ROWS_AT_PIECE[(_r * NT_S + NFR) // PIECE].append(_r)

ASL_CHUNKS = 8          # ampsel DMA split for early pipeline start
# staging-copy engine cycle: g=gpsimd(Pool), v=vector(DVE), a=scalar(Act)
COPY_CYCLE = "a"
OUT_GROUP = 4           # rows per output DMA

_BUILT = None


def _build_consts():
    import ml_dtypes
    hann = np.hanning(WIN)
    k = np.arange(CBINS)[:, None]
    n = np.arange(WIN)[None, :]
    ang = 2.0 * np.pi * k * n / WIN
    Cm = np.cos(ang) / WIN * np.where((k >= 1) & (k <= 255), 2.0, 1.0)
    Sm = -np.sin(ang) / WIN * np.where((k >= 1) & (k <= 255), 2.0, 0.0)
    Cw = Cm * hann[None, :]
    Sw = Sm * hann[None, :]
    W1 = np.concatenate([Cw[:256, :STEP], Sw[:256, :STEP]], 0)   # (512,256)
    W2 = np.concatenate([Cw[:256, STEP:], Sw[:256, STEP:]], 0)   # (512,256)
    wm = np.zeros((128, 8 * 256))
    for j in range(4):
        wm[:, 256 * j:256 * (j + 1)] = W1[128 * j:128 * (j + 1), :]
        wm[:, 256 * (j + 4):256 * (j + 5)] = W2[128 * j:128 * (j + 1), :]
    # bin 256 is added on the host (rank-1 per row); the im-bin-0 rows of
    # the Sw chunks stay zero, so IM[0] partition 0 can hold anything.
    wm_bf = wm.astype(ml_dtypes.bfloat16)
    w256 = Cw[256, :]                                # (512,) f64 basis

    # every row's t=0 slot produces RE = -1 (amp=1, cos arg = -pi) across all
    # 256 bins; its W2 contribution (chunks 4,5) is a constant per-sample
    # vector cancelled on host after the device run.
    wm_f64 = wm_bf.astype(np.float64)
    corr = (wm_f64[:, 4 * 256:5 * 256].sum(0)
            + wm_f64[:, 5 * 256:6 * 256].sum(0)).astype(np.float32)

    t = np.arange(NT, dtype=np.float64)
    sel_t = np.zeros((64, FREE))
    sel_1 = np.zeros((64, FREE))
    for r in range(64):
        sel_t[r, r * NT_S:r * NT_S + NT] = t
        sel_1[r, r * NT_S + 1:r * NT_S + NT] = 1.0   # t=0 slot stays 0
    ampsel = np.concatenate([sel_t, sel_1], 0)       # (128,8704)
    return (wm_bf, ampsel.astype(np.float32), corr, w256)


def _kernel_body(ctx: ExitStack, tc, out_d, sel_d, items_d, wm_d,
                 ampsel_d, ident_d, negi_d):
    import concourse.mybir as mybir

    nc = tc.nc
    f32 = mybir.dt.float32
    f32r = mybir.dt.float32r
    f16 = mybir.dt.float16
    bf16 = mybir.dt.bfloat16
    AF = mybir.ActivationFunctionType
    OP = mybir.AluOpType
    PI = math.pi
    MAGIC = 12582912.0

    const = ctx.enter_context(tc.tile_pool(name="const", bufs=1))
    persist = ctx.enter_context(tc.tile_pool(name="persist", bufs=1))

    _pn = [0]

    def p2tile(shape, name):
        _pn[0] += 1
        return pst2.tile(shape, f32, tag="p2", name=f"{name}{_pn[0]}")

    # scoped pools: ampsel (dead after phase 1) and stage-1/2 transients
    # (dead after stage 2); released zones are reused by phase-2 pools.
    # Created in reverse release order (pool stack is LIFO).
    ppam = tc.tile_pool(name="ppam", bufs=2, space="PSUM")
    ppa = ppam.__enter__()
    pst2p = tc.tile_pool(name="pst2", bufs=2, space="PSUM")
    pst2 = pst2p.__enter__()
    s2p = tc.tile_pool(name="s2p", bufs=1)
    s2 = s2p.__enter__()

    # ---- constants + inputs (SP DMA queue is in-order: stage-2 inputs
    # first, then ampsel chunks, wm last — first needed only at first DFT)
    it4 = s2.tile([128, 4 * N_COEFFS + 4], f32)
    items_v = items_d.rearrange("(a p) c -> p a c", p=128)
    nc.sync.dma_start(it4[:, 0:4 * N_COEFFS].rearrange("p (a c) -> p a c", a=4),
                      items_v)
    zpad = s2.tile([128, 4], f32)
    nc.vector.memset(zpad[:], 0.0)
    nc.vector.tensor_copy(it4[:, 4 * N_COEFFS:], zpad[:])
    sel_t = s2.tile([64, 512], f32)
    nc.sync.dma_start(sel_t[:], sel_d[:])
    ident = const.tile([128, 128], f32)
    nc.sync.dma_start(ident[:], ident_d[:])
    asl = const.tile([128, FREE], f32r)
    ck = FREE // ASL_CHUNKS
    for i in range(ASL_CHUNKS):
        nc.sync.dma_start(asl[:, i * ck:(i + 1) * ck],
                          ampsel_d[:, i * ck:(i + 1) * ck])
    wm = const.tile([128, 2048], bf16)
    nc.sync.dma_start(wm[:], wm_d[:])
    bias_half = const.tile([128, 1], f32)
    nc.vector.memset(bias_half[:], 0.5)
    bias_npi = const.tile([128, 1], f32)
    nc.vector.memset(bias_npi[:], -PI)
    zeros64 = const.tile([64, 128], f32)
    nc.vector.memset(zeros64[:], 0.0)
    quart64 = const.tile([64, 128], f32)
    nc.vector.memset(quart64[:], 0.25)
    negi = const.tile([128, 128], f16)
    nc.sync.dma_start(negi[:], negi_d[:])

    rs = s2.tile([64, 512], f32)
    nc.scalar.activation(rs[:], sel_t[:], AF.Relu)

    rsT = []
    for kc in range(4):
        pt_ = p2tile([128, 64], "tr")
        nc.tensor.transpose(pt_[:], rs[:, kc * 128:(kc + 1) * 128],
                            ident[0:64, 0:64])
        st = s2.tile([128, 64], f32, name=f"rsT{kc}")
        nc.vector.tensor_copy(st[:], pt_[:])
        rsT.append(st)

    coefA = s2.tile([64, N_COEFFS], f32)
    pA1 = p2tile([64, 512], "pA")
    for kc in range(4):
        nc.tensor.matmul(pA1[:], rsT[kc][:], it4[:, kc * N_COEFFS:kc * N_COEFFS + 512],
                         start=(kc == 0), stop=(kc == 3))
    pA2 = p2tile([64, 260], "pA")
    for kc in range(4):
        # 260-wide (f32r needs even free size); col 259 is padding/garbage
        nc.tensor.matmul(pA2[:], rsT[kc][:],
                         it4[:, kc * N_COEFFS + 512:kc * N_COEFFS + 772],
                         start=(kc == 0), stop=(kc == 3))
    nc.vector.tensor_copy(coefA[:, 0:512], pA1[:])
    nc.vector.tensor_copy(coefA[:, 512:771], pA2[:, 0:259])

    # ---- stage 2: activation blocks ----
    # block A: sigmoid table (sigmoid + tanh)
    sig_mag, sig_st, th = {}, {}, {}
    for c in range(2):
        sm = s2.tile([64, 128], f32, name=f"sigmag{c}")
        nc.scalar.activation(sm[:], coefA[:, c * 128:(c + 1) * 128], AF.Sigmoid)
        sig_mag[c] = sm
        ss = s2.tile([64, 128], f32, name=f"sigst{c}")
        nc.scalar.activation(ss[:], coefA[:, 2 * CBINS + c * 128:2 * CBINS + (c + 1) * 128],
                             AF.Sigmoid)
        sig_st[c] = ss
        tt = s2.tile([64, 128], f32, name=f"th{c}")
        nc.scalar.activation(tt[:], coefA[:, CBINS + c * 128:CBINS + (c + 1) * 128],
                             AF.Tanh)
        th[c] = tt

    # weight tiles for the arg matmuls
    ampx, thT, thTc = {}, {}, {}
    for c in range(2):
        ax = persist.tile([128, 128], f32r, name=f"ampx{c}")
        ampx[c] = ax
        tx = persist.tile([128, 128], f32r, name=f"thT{c}")
        nc.vector.tensor_copy(tx[64:128, :], zeros64[:])
        nc.vector.tensor_scalar(tx[0:64, :], th[c][:], 0.5, None, OP.mult)
        thT[c] = tx
        txc = persist.tile([128, 128], f32r, name=f"thTc{c}")
        nc.vector.tensor_copy(txc[64:128, :], quart64[:])
        nc.vector.tensor_copy(txc[0:64, :], tx[0:64, :])
        thTc[c] = txc

    # block B: natural_log_exp table (Ln now, Exp pieces later share it).
    # The fence keeps the scheduler from interleaving sigmoid/tanh with Ln,
    # which would thrash activation tables.
    tc.no_sync_barrier()
    for c in range(2):
        lgm = s2.tile([64, 128], f32, name=f"lgm{c}")
        nc.scalar.activation(lgm[:], sig_mag[c][:], AF.Ln,
                             bias=bias_half[0:64], scale=0.49995)
        nc.vector.tensor_copy(ampx[c][0:64, :], lgm[:])
        lgs = s2.tile([64, 128], f32, name=f"lgs{c}")
        nc.scalar.activation(lgs[:], sig_st[c][:], AF.Ln)
        nc.vector.tensor_copy(ampx[c][64:128, :], lgs[:])
    s2p.__exit__(None, None, None)
    pst2p.__exit__(None, None, None)

    # ---- stage 3+4: two phases (exp+mods, then sin+DFT) ----
    AMP = [persist.tile([128, FREE], bf16, name=f"AMP{i}") for i in range(2)]

    specs = [None, None, None, None]
    cp_i = [0]
    ost_state = {"tile": None, "po": None}
    pending = []            # completed po pair tiles awaiting staging copy

    def emit_row_mm(r):
        # two rows share one PSUM bank; the pair joins `pending` on the
        # odd row and is staged a couple of pieces later (so the staging
        # copy never heads-of-line-blocks the DVE queue behind PE)
        half = r % 2
        if half == 0:
            _pn[0] += 1
            ost_state["po"] = ppo.tile([128, 512], f32, tag="po",
                                       name=f"po{_pn[0]}")
        po = ost_state["po"]
        pv = po[:, half * 256:(half + 1) * 256]
        for j in range(4):
            nc.tensor.matmul(pv,
                             specs[j][:, r * NT_S + 1:r * NT_S + NT],
                             wm[:, j * 256:(j + 1) * 256],
                             start=(j == 0), stop=False)
        for j in range(4):
            nc.tensor.matmul(pv,
                             specs[j][:, r * NT_S:r * NT_S + NFR],
                             wm[:, (j + 4) * 256:(j + 5) * 256],
                             start=False, stop=(j == 3))
        if half == 1:
            pending.append((r - 1, po))

    def flush_pairs(n=None):
        k = len(pending) if n is None else min(n, len(pending))
        for _ in range(k):
            r0p, po = pending.pop(0)
            slot = r0p % OUT_GROUP
            if slot == 0:
                ost_state["tile"] = ostage.tile(
                    [128, OUT_GROUP * 256], f32, tag="ost",
                    name=f"ost{r0p // OUT_GROUP}")
            ost = ost_state["tile"]
            dst = ost[:, slot * 256:(slot + 2) * 256]
            eng = COPY_CYCLE[cp_i[0] % len(COPY_CYCLE)]
            cp_i[0] += 1
            if eng == "v":
                nc.vector.tensor_copy(dst, po[:])
            else:
                nc.scalar.copy(dst, po[:])
            if slot == OUT_GROUP - 2:
                r0 = r0p - OUT_GROUP + 2
                dstv = out_d[r0:r0 + OUT_GROUP, :].rearrange(
                    "r (f s) -> f r s", f=NFR)
                nc.sync.dma_start(dstv, ost[:].rearrange(
                    "f (r s) -> f r s", r=OUT_GROUP))

    # --- phase 1: arg matmuls + Exp activations (Ln/Exp share a table) ---
    tc.no_sync_barrier()
    for p in range(NPIECE):
        w0, w1 = p * PIECE, (p + 1) * PIECE
        for c in range(2):
            _pn[0] += 1
            pa = ppa.tile([128, PIECE], f32, tag="pa", name=f"pa{_pn[0]}")
            nc.tensor.matmul(pa[:], ampx[c][:], asl[:, w0:w1],
                             start=True, stop=True)
            nc.scalar.activation(AMP[c][:, w0:w1], pa[:], AF.Exp)
    ppam.__exit__(None, None, None)

    # --- phase 2: theta matmul pairs, magic-round + negi subtract (u stays
    # f32 in PSUM), one Sin per unit, spec products, DFT + output.
    pptp = ctx.enter_context(tc.tile_pool(name="pptp", bufs=3, space="PSUM"))
    ppo = ctx.enter_context(tc.tile_pool(name="ppo", bufs=2, space="PSUM"))
    reim = ctx.enter_context(tc.tile_pool(name="reim", bufs=1))
    rspool = ctx.enter_context(tc.tile_pool(name="rspool", bufs=4))
    scpool = ctx.enter_context(tc.tile_pool(name="scpool", bufs=4))
    ostage = ctx.enter_context(tc.tile_pool(name="ostage", bufs=2))
    RE = [reim.tile([128, FREE], bf16, name=f"RE{i}") for i in range(2)]
    IM = [reim.tile([128, FREE], bf16, name=f"IM{i}") for i in range(2)]
    specs[:] = [RE[0], RE[1], IM[0], IM[1]]

    units = [(p, c) for p in range(NPIECE) for c in range(2)]
    ustate = {}

    def emit_ptmm_rs(i):
        p, c = units[i]
        w0, w1 = p * PIECE, (p + 1) * PIECE
        _pn[0] += 1
        ptp = pptp.tile([128, 2 * PIECE], f32, tag="ptp", name=f"ptp{_pn[0]}")
        nc.tensor.matmul(ptp[:, 0:PIECE], thT[c][:], asl[:, w0:w1],
                         start=True, stop=True)
        nc.tensor.matmul(ptp[:, PIECE:], thTc[c][:], asl[:, w0:w1],
                         start=True, stop=True)
        rs_ = rspool.tile([128, 2 * PIECE], f16, tag="rs", name=f"rs{_pn[0]}")
        nc.vector.tensor_scalar(rs_[:], ptp[:], MAGIC, MAGIC,
                                OP.add, OP.subtract)
        ustate[i] = (ptp, rs_)

    def emit_unit_tail(i):
        p, c = units[i]
        w0, w1 = p * PIECE, (p + 1) * PIECE
        ptp, rs_ = ustate.pop(i)
        nc.tensor.matmul(ptp[:, 0:PIECE], negi[:], rs_[:, 0:PIECE],
                         start=False, stop=True, skip_group_check=True)
        nc.tensor.matmul(ptp[:, PIECE:], negi[:], rs_[:, PIECE:],
                         start=False, stop=True, skip_group_check=True)
        _pn[0] += 1
        sc = scpool.tile([128, 2 * PIECE], bf16, tag="sc", name=f"sc{_pn[0]}")
        nc.scalar.activation(sc[:], ptp[:], AF.Sin, scale=2.0 * PI)
        # sc[:, :PIECE] = sin, sc[:, PIECE:] = cos
        nc.vector.tensor_tensor(RE[c][:, w0:w1], AMP[c][:, w0:w1],
                                sc[:, PIECE:], OP.mult)
        nc.vector.tensor_tensor(IM[c][:, w0:w1], AMP[c][:, w0:w1],
                                sc[:, 0:PIECE], OP.mult)

    tc.no_sync_barrier()
    emit_ptmm_rs(0)
    for i in range(len(units)):
        p, c = units[i]
        if i + 1 < len(units):
            emit_ptmm_rs(i + 1)
        emit_unit_tail(i)
        if c == 1:
            for r in ROWS_AT_PIECE[p]:
                emit_row_mm(r)
                flush_pairs()
    flush_pairs()


def _build():
    global _BUILT
    if _BUILT is not None:
        return _BUILT
    import concourse.bacc as bacc
    import concourse.mybir as mybir
    import concourse.tile as tile

    wm_np, ampsel_np, corr_np, w256_np = _build_consts()
    ident_np = np.eye(128, dtype=np.float32)
    negi_np = -np.eye(128, dtype=np.float16)

    nc = bacc.Bacc("TRN2", target_bir_lowering=False, debug=False,
                   num_devices=N_CORES)
    f32 = mybir.dt.float32
    bf16 = mybir.dt.bfloat16
    sel_d = nc.dram_tensor("sel", [R_PER_CORE, N_ITEMS], f32,
                           kind="ExternalInput").ap()
    items_d = nc.dram_tensor("items", [N_ITEMS, N_COEFFS],
                             f32, kind="ExternalInput").ap()
    wm_d = nc.dram_tensor("wm", list(wm_np.shape), bf16,
                          kind="ExternalInput").ap()
    ampsel_d = nc.dram_tensor("ampsel", list(ampsel_np.shape),
                              mybir.dt.float32r, kind="ExternalInput").ap()
    ident_d = nc.dram_tensor("ident", [128, 128], f32,
                             kind="ExternalInput").ap()
    negi_d = nc.dram_tensor("negi", [128, 128], mybir.dt.float16,
                            kind="ExternalInput").ap()
    out_d = nc.dram_tensor("out", [R_PER_CORE, NFR * STEP], f32,
                           kind="ExternalOutput").ap()

    with tile.TileContext(nc) as tc:
        with ExitStack() as ctx:
            _kernel_body(ctx, tc, out_d, sel_d, items_d, wm_d,
                         ampsel_d, ident_d, negi_d)
    nc.compile()

    _BUILT = (nc, wm_np, ampsel_np, ident_np, negi_np, corr_np, w256_np)
    return _BUILT


def _in_maps(selections, items, wm_np, ampsel_np, ident_np, negi_np):
    sel_flat = np.ascontiguousarray(
        np.asarray(selections).reshape(NROWS, N_ITEMS).astype(np.float32))
    items_f = np.ascontiguousarray(np.asarray(items).astype(np.float32))
    maps = []
    for c in range(N_CORES):
        maps.append({
            "sel": sel_flat[c * R_PER_CORE:(c + 1) * R_PER_CORE],
            "items": items_f,
            "wm": wm_np,
            "ampsel": ampsel_np,
            "ident": ident_np,
            "negi": negi_np,
        })
    return maps


def _bin256_host(selections, items, w256):
    # bin 256 (Nyquist) spectral line, computed exactly on host: per row a
    # rank-1 [frames x window] contribution folded with hann + overlap-add
    sel = np.maximum(np.asarray(selections, np.float64).reshape(NROWS, N_ITEMS),
                     0.0)
    it = np.asarray(items, np.float64)[:, [256, 256 + CBINS, 256 + 2 * CBINS]]
    c3 = sel @ it                                            # (512, 3)
    mag = 0.5 + 1.0 / (1.0 + np.exp(-c3[:, 0])) * 0.49995
    ph = np.tanh(c3[:, 1]) * np.pi
    st = 1.0 / (1.0 + np.exp(-c3[:, 2]))
    t = np.arange(NT, dtype=np.float64)                      # 0..128
    spec = (st[:, None] * np.exp(np.log(mag + 1e-12)[:, None] * t)
            * np.cos(ph[:, None] * t))                       # (512, 129)
    sw1 = spec[:, 1:NT]                                      # t = f+1
    sw2 = spec[:, 0:NFR].copy()
    sw2[:, 0] = 0.0                                          # no frame -1
    return (sw1[:, :, None] * w256[None, None, :STEP]
            + sw2[:, :, None] * w256[None, None, STEP:])     # (512,128,256)


def kernel(selections: np.ndarray, items: np.ndarray) -> np.ndarray:
    from concourse.bass_utils import run_bass_kernel_spmd

    nc, wm_np, ampsel_np, ident_np, negi_np, corr_np, w256_np = _build()
    in_maps = _in_maps(selections, items, wm_np, ampsel_np, ident_np, negi_np)
    res = run_bass_kernel_spmd(nc, in_maps, core_ids=list(range(N_CORES)))
    rows = np.concatenate([res.results[c]["out"] for c in range(N_CORES)], 0)
    rows = rows.astype(np.float64)
    # t=0 slot: theta-const rides sel_1 (zeroed at t0), so cos(t0)=sin(0)=0
    # and RE(t0)=0 on device already -- no correction needed
    rows += _bin256_host(selections, items, w256_np).reshape(NROWS, -1)
    norms = np.linalg.norm(rows, axis=-1, keepdims=True)
    rows = rows / (norms + 1e-8)
    sh = np.asarray(selections).shape
    return rows.reshape(sh[0], sh[1], sh[2], NFR * STEP).astype(np.float32)


# revision 93
# speedup vs baseline: 1.0730x; 1.0252x over previous
"""Trainium2 Bass kernel for nn_FFTResonanceLookup.

Full inputs: selections (8,16,4,512) f32, items (512,771) f32.
Output: (8,16,4,32768) f32, unit-normalized along samples.

Data-parallel over the 512 (b,e,x) rows -> 64 rows/core x 8 cores.
Per row the synth is a 1024-feature x 256-sample matmul (irfft + hann
window + 50%-overlap-add folded into constant weights). Two phases so
the Act engine never thrashes activation tables: (1) all arg matmuls +
Exp (Ln/Exp share a table), (2) theta matmul pairs with magic-number
round + negi-matmul subtract (reduced phase stays f32 in PSUM), one
Sin per unit, spec products, and per-row DFT matmuls interleaved as
spec pieces complete. Bin 256 (Nyquist) is a rank-1 per-row term added
on host in f64; normalization is also on host.
"""

import math
from contextlib import ExitStack

import numpy as np

N_ITEMS = 512
N_COEFFS = 771
CBINS = 257
WIN = 512
STEP = 256
NFR = 128
NT = NFR + 1            # t = 0..128
NT_S = 136              # padded per-row stride: 64*136 = 8704 = 17*512
R_PER_CORE = 64
N_CORES = 8
NROWS = 512
PIECE = 512
NPIECE = 17
FREE = R_PER_CORE * NT_S  # 8704

# piece p -> rows whose DFT windows are fully covered once piece p done
ROWS_AT_PIECE = [[] for _ in range(NPIECE)]
for _r in range(R_PER_CORE):
    ROWS_AT_PIECE[(_r * NT_S + NFR) // PIECE].append(_r)

ASL_CHUNKS = 8          # ampsel DMA split for early pipeline start
# staging-copy engine cycle: g=gpsimd(Pool), v=vector(DVE), a=scalar(Act)
COPY_CYCLE = "a"
OUT_GROUP = 4           # rows per output DMA

_BUILT = None


def _build_consts():
    import ml_dtypes
    hann = np.hanning(WIN)
    k = np.arange(CBINS)[:, None]
    n = np.arange(WIN)[None, :]
    ang = 2.0 * np.pi * k * n / WIN
    Cm = np.cos(ang) / WIN * np.where((k >= 1) & (k <= 255), 2.0, 1.0)
    Sm = -np.sin(ang) / WIN * np.where((k >= 1) & (k <= 255), 2.0, 0.0)
    Cw = Cm * hann[None, :]
    Sw = Sm * hann[None, :]
    W1 = np.concatenate([Cw[:256, :STEP], Sw[:256, :STEP]], 0)   # (512,256)
    W2 = np.concatenate([Cw[:256, STEP:], Sw[:256, STEP:]], 0)   # (512,256)
    wm = np.zeros((128, 8 * 256))
    for j in range(4):
        wm[:, 256 * j:256 * (j + 1)] = W1[128 * j:128 * (j + 1), :]
        wm[:, 256 * (j + 4):256 * (j + 5)] = W2[128 * j:128 * (j + 1), :]
    # bin 256 is added on the host (rank-1 per row); the im-bin-0 rows of
    # the Sw chunks stay zero, so IM[0] partition 0 can hold anything.
    wm_bf = wm.astype(ml_dtypes.bfloat16)
    w256 = Cw[256, :]                                # (512,) f64 basis

    # every row's t=0 slot produces RE = -1 (amp=1, cos arg = -pi) across all
    # 256 bins; its W2 contribution (chunks 4,5) is a constant per-sample
    # vector cancelled on host after the device run.
    wm_f64 = wm_bf.astype(np.float64)
    corr = (wm_f64[:, 4 * 256:5 * 256].sum(0)
            + wm_f64[:, 5 * 256:6 * 256].sum(0)).astype(np.float32)

    t = np.arange(NT, dtype=np.float64)
    sel_t = np.zeros((64, FREE))
    sel_1 = np.zeros((64, FREE))
    for r in range(64):
        sel_t[r, r * NT_S:r * NT_S + NT] = t
        sel_1[r, r * NT_S + 1:r * NT_S + NT] = 1.0   # t=0 slot stays 0
    ampsel = np.concatenate([sel_t, sel_1], 0)       # (128,8704)
    return (wm_bf, ampsel.astype(np.float32), corr, w256)


def _kernel_body(ctx: ExitStack, tc, out_d, sel_d, items_d, wm_d,
                 ampsel_d, ident_d, negi_d):
    import concourse.mybir as mybir

    nc = tc.nc
    f32 = mybir.dt.float32
    f32r = mybir.dt.float32r
    f16 = mybir.dt.float16
    bf16 = mybir.dt.bfloat16
    AF = mybir.ActivationFunctionType
    OP = mybir.AluOpType
    PI = math.pi
    MAGIC = 12582912.0

    const = ctx.enter_context(tc.tile_pool(name="const", bufs=1))
    persist = ctx.enter_context(tc.tile_pool(name="persist", bufs=1))

    _pn = [0]

    def p2tile(shape, name):
        _pn[0] += 1
        return pst2.tile(shape, f32, tag="p2", name=f"{name}{_pn[0]}")

    # scoped pools: ampsel (dead after phase 1) and stage-1/2 transients
    # (dead after stage 2); released zones are reused by phase-2 pools.
    # Created in reverse release order (pool stack is LIFO).
    ppam = tc.tile_pool(name="ppam", bufs=2, space="PSUM")
    ppa = ppam.__enter__()
    pst2p = tc.tile_pool(name="pst2", bufs=2, space="PSUM")
    pst2 = pst2p.__enter__()
    s2p = tc.tile_pool(name="s2p", bufs=1)
    s2 = s2p.__enter__()

    # ---- constants + inputs (SP DMA queue is in-order: stage-2 inputs
    # first, then ampsel chunks, wm last — first needed only at first DFT)
    it4 = s2.tile([128, 4 * N_COEFFS + 4], f32)
    items_v = items_d.rearrange("(a p) c -> p a c", p=128)
    nc.sync.dma_start(it4[:, 0:4 * N_COEFFS].rearrange("p (a c) -> p a c", a=4),
                      items_v)
    zpad = s2.tile([128, 4], f32)
    nc.vector.memset(zpad[:], 0.0)
    nc.vector.tensor_copy(it4[:, 4 * N_COEFFS:], zpad[:])
    sel_t = s2.tile([64, 512], f32)
    nc.sync.dma_start(sel_t[:], sel_d[:])
    ident = const.tile([128, 128], f32)
    nc.sync.dma_start(ident[:], ident_d[:])
    asl = const.tile([128, FREE], f32r)
    ck = FREE // ASL_CHUNKS
    for i in range(ASL_CHUNKS):
        nc.sync.dma_start(asl[:, i * ck:(i + 1) * ck],
                          ampsel_d[:, i * ck:(i + 1) * ck])
    wm = const.tile([128, 2048], bf16)
    nc.sync.dma_start(wm[:], wm_d[:])
    bias_half = const.tile([128, 1], f32)
    nc.vector.memset(bias_half[:], 0.5)
    bias_npi = const.tile([128, 1], f32)
    nc.vector.memset(bias_npi[:], -PI)
    zeros64 = const.tile([64, 128], f32)
    nc.vector.memset(zeros64[:], 0.0)
    quart64 = const.tile([64, 128], f32)
    nc.vector.memset(quart64[:], 0.25)
    negi = const.tile([128, 128], f16)
    nc.sync.dma_start(negi[:], negi_d[:])

    rs = s2.tile([64, 512], f32)
    nc.scalar.activation(rs[:], sel_t[:], AF.Relu)

    rsT = []
    for kc in range(4):
        pt_ = p2tile([128, 64], "tr")
        nc.tensor.transpose(pt_[:], rs[:, kc * 128:(kc + 1) * 128],
                            ident[0:64, 0:64])
        st = s2.tile([128, 64], f32, name=f"rsT{kc}")
        nc.vector.tensor_copy(st[:], pt_[:])
        rsT.append(st)

    coefA = s2.tile([64, N_COEFFS], f32)
    pA1 = p2tile([64, 512], "pA")
    for kc in range(4):
        nc.tensor.matmul(pA1[:], rsT[kc][:], it4[:, kc * N_COEFFS:kc * N_COEFFS + 512],
                         start=(kc == 0), stop=(kc == 3))
    pA2 = p2tile([64, 260], "pA")
    for kc in range(4):
        # 260-wide (f32r needs even free size); col 259 is padding/garbage
        nc.tensor.matmul(pA2[:], rsT[kc][:],
                         it4[:, kc * N_COEFFS + 512:kc * N_COEFFS + 772],
                         start=(kc == 0), stop=(kc == 3))
    nc.vector.tensor_copy(coefA[:, 0:512], pA1[:])
    nc.vector.tensor_copy(coefA[:, 512:771], pA2[:, 0:259])

    # ---- stage 2: activation blocks ----
    # block A: sigmoid table (sigmoid + tanh)
    sig_mag, sig_st, th = {}, {}, {}
    for c in range(2):
        sm = s2.tile([64, 128], f32, name=f"sigmag{c}")
        nc.scalar.activation(sm[:], coefA[:, c * 128:(c + 1) * 128], AF.Sigmoid)
        sig_mag[c] = sm
        ss = s2.tile([64, 128], f32, name=f"sigst{c}")
        nc.scalar.activation(ss[:], coefA[:, 2 * CBINS + c * 128:2 * CBINS + (c + 1) * 128],
                             AF.Sigmoid)
        sig_st[c] = ss
        tt = s2.tile([64, 128], f32, name=f"th{c}")
        nc.scalar.activation(tt[:], coefA[:, CBINS + c * 128:CBINS + (c + 1) * 128],
                             AF.Tanh)
        th[c] = tt

    # weight tiles for the arg matmuls
    ampx, thT, thTc = {}, {}, {}
    for c in range(2):
        ax = persist.tile([128, 128], f32r, name=f"ampx{c}")
        ampx[c] = ax
        tx = persist.tile([128, 128], f32r, name=f"thT{c}")
        nc.vector.tensor_copy(tx[64:128, :], zeros64[:])
        nc.vector.tensor_scalar(tx[0:64, :], th[c][:], 0.5, None, OP.mult)
        thT[c] = tx
        txc = persist.tile([128, 128], f32r, name=f"thTc{c}")
        nc.vector.tensor_copy(txc[64:128, :], quart64[:])
        nc.vector.tensor_copy(txc[0:64, :], tx[0:64, :])
        thTc[c] = txc

    # block B: natural_log_exp table (Ln now, Exp pieces later share it).
    # The fence keeps the scheduler from interleaving sigmoid/tanh with Ln,
    # which would thrash activation tables.
    tc.no_sync_barrier()
    for c in range(2):
        lgm = s2.tile([64, 128], f32, name=f"lgm{c}")
        nc.scalar.activation(lgm[:], sig_mag[c][:], AF.Ln,
                             bias=bias_half[0:64], scale=0.49995)
        nc.vector.tensor_copy(ampx[c][0:64, :], lgm[:])
        lgs = s2.tile([64, 128], f32, name=f"lgs{c}")
        nc.scalar.activation(lgs[:], sig_st[c][:], AF.Ln)
        nc.vector.tensor_copy(ampx[c][64:128, :], lgs[:])
    s2p.__exit__(None, None, None)
    pst2p.__exit__(None, None, None)

    # ---- stage 3+4: two phases (exp+mods, then sin+DFT) ----
    AMP = [persist.tile([128, FREE], bf16, name=f"AMP{i}") for i in range(2)]

    specs = [None, None, None, None]
    cp_i = [0]
    ost_state = {"tile": None, "po": None}
    pending = []            # completed po pair tiles awaiting staging copy

    def emit_row_mm(r):
        # two rows share one PSUM bank; the pair joins `pending` on the
        # odd row and is staged a couple of pieces later (so the staging
        # copy never heads-of-line-blocks the DVE queue behind PE)
        half = r % 2
        if half == 0:
            _pn[0] += 1
            ost_state["po"] = ppo.tile([128, 512], f32, tag="po",
                                       name=f"po{_pn[0]}")
        po = ost_state["po"]
        pv = po[:, half * 256:(half + 1) * 256]
        for j in range(4):
            nc.tensor.matmul(pv,
                             specs[j][:, r * NT_S + 1:r * NT_S + NT],
                             wm[:, j * 256:(j + 1) * 256],
                             start=(j == 0), stop=False)
        for j in range(4):
            nc.tensor.matmul(pv,
                             specs[j][:, r * NT_S:r * NT_S + NFR],
                             wm[:, (j + 4) * 256:(j + 5) * 256],
                             start=False, stop=(j == 3))
        if half == 1:
            pending.append((r - 1, po))

    def flush_pairs(n=None):
        k = len(pending) if n is None else min(n, len(pending))
        for _ in range(k):
            r0p, po = pending.pop(0)
            slot = r0p % OUT_GROUP
            if slot == 0:
                ost_state["tile"] = ostage.tile(
                    [128, OUT_GROUP * 256], f32, tag="ost",
                    name=f"ost{r0p // OUT_GROUP}")
            ost = ost_state["tile"]
            dst = ost[:, slot * 256:(slot + 2) * 256]
            eng = COPY_CYCLE[cp_i[0] % len(COPY_CYCLE)]
            cp_i[0] += 1
            if eng == "v":
                nc.vector.tensor_copy(dst, po[:])
            else:
                nc.scalar.copy(dst, po[:])
            if slot == OUT_GROUP - 2:
                r0 = r0p - OUT_GROUP + 2
                dstv = out_d[r0:r0 + OUT_GROUP, :].rearrange(
                    "r (f s) -> f r s", f=NFR)
                nc.sync.dma_start(dstv, ost[:].rearrange(
                    "f (r s) -> f r s", r=OUT_GROUP))

    # --- phase 1: arg matmuls + Exp activations (Ln/Exp share a table) ---
    tc.no_sync_barrier()
    for p0 in range(0, NPIECE - 1, 2):
        w0 = p0 * PIECE
        for c in range(2):
            _pn[0] += 1
            pa = ppa.tile([128, 2 * PIECE], f32, tag="pa", name=f"pa{_pn[0]}")
            nc.tensor.matmul(pa[:, 0:PIECE], ampx[c][:],
                             asl[:, w0:w0 + PIECE], start=True, stop=True)
            nc.tensor.matmul(pa[:, PIECE:], ampx[c][:],
                             asl[:, w0 + PIECE:w0 + 2 * PIECE],
                             start=True, stop=True)
            nc.scalar.activation(AMP[c][:, w0:w0 + 2 * PIECE], pa[:], AF.Exp)
    w0 = (NPIECE - 1) * PIECE
    for c in range(2):
        _pn[0] += 1
        pa = ppa.tile([128, PIECE], f32, tag="pa", name=f"pa{_pn[0]}")
        nc.tensor.matmul(pa[:], ampx[c][:], asl[:, w0:w0 + PIECE],
                         start=True, stop=True)
        nc.scalar.activation(AMP[c][:, w0:w0 + PIECE], pa[:], AF.Exp)
    ppam.__exit__(None, None, None)

    # --- phase 2: theta matmul pairs, magic-round + negi subtract (u stays
    # f32 in PSUM), one Sin per unit, spec products, DFT + output.
    pptp = ctx.enter_context(tc.tile_pool(name="pptp", bufs=3, space="PSUM"))
    ppo = ctx.enter_context(tc.tile_pool(name="ppo", bufs=2, space="PSUM"))
    reim = ctx.enter_context(tc.tile_pool(name="reim", bufs=1))
    rspool = ctx.enter_context(tc.tile_pool(name="rspool", bufs=4))
    scpool = ctx.enter_context(tc.tile_pool(name="scpool", bufs=4))
    ostage = ctx.enter_context(tc.tile_pool(name="ostage", bufs=2))
    RE = [reim.tile([128, FREE], bf16, name=f"RE{i}") for i in range(2)]
    IM = [reim.tile([128, FREE], bf16, name=f"IM{i}") for i in range(2)]
    specs[:] = [RE[0], RE[1], IM[0], IM[1]]

    units = [(p, c) for p in range(NPIECE) for c in range(2)]
    ustate = {}

    def emit_ptmm_rs(i):
        p, c = units[i]
        w0, w1 = p * PIECE, (p + 1) * PIECE
        _pn[0] += 1
        ptp = pptp.tile([128, 2 * PIECE], f32, tag="ptp", name=f"ptp{_pn[0]}")
        nc.tensor.matmul(ptp[:, 0:PIECE], thT[c][:], asl[:, w0:w1],
                         start=True, stop=True)
        nc.tensor.matmul(ptp[:, PIECE:], thTc[c][:], asl[:, w0:w1],
                         start=True, stop=True)
        rs_ = rspool.tile([128, 2 * PIECE], f16, tag="rs", name=f"rs{_pn[0]}")
        nc.vector.tensor_scalar(rs_[:], ptp[:], MAGIC, MAGIC,
                                OP.add, OP.subtract)
        ustate[i] = (ptp, rs_)

    def emit_unit_tail(i):
        p, c = units[i]
        w0, w1 = p * PIECE, (p + 1) * PIECE
        ptp, rs_ = ustate.pop(i)
        nc.tensor.matmul(ptp[:, 0:PIECE], negi[:], rs_[:, 0:PIECE],
                         start=False, stop=True, skip_group_check=True)
        nc.tensor.matmul(ptp[:, PIECE:], negi[:], rs_[:, PIECE:],
                         start=False, stop=True, skip_group_check=True)
        _pn[0] += 1
        sc = scpool.tile([128, 2 * PIECE], bf16, tag="sc", name=f"sc{_pn[0]}")
        nc.scalar.activation(sc[:], ptp[:], AF.Sin, scale=2.0 * PI)
        # sc[:, :PIECE] = sin, sc[:, PIECE:] = cos
        nc.vector.tensor_tensor(RE[c][:, w0:w1], AMP[c][:, w0:w1],
                                sc[:, PIECE:], OP.mult)
        nc.vector.tensor_tensor(IM[c][:, w0:w1], AMP[c][:, w0:w1],
                                sc[:, 0:PIECE], OP.mult)

    tc.no_sync_barrier()
    emit_ptmm_rs(0)
    for i in range(len(units)):
        p, c = units[i]
        if i + 1 < len(units):
            emit_ptmm_rs(i + 1)
        emit_unit_tail(i)
        if c == 1:
            for r in ROWS_AT_PIECE[p]:
                emit_row_mm(r)
                flush_pairs()
    flush_pairs()


def _build():
    global _BUILT
    if _BUILT is not None:
        return _BUILT
    import concourse.bacc as bacc
    import concourse.mybir as mybir
    import concourse.tile as tile

    wm_np, ampsel_np, corr_np, w256_np = _build_consts()
    ident_np = np.eye(128, dtype=np.float32)
    negi_np = -np.eye(128, dtype=np.float16)

    nc = bacc.Bacc("TRN2", target_bir_lowering=False, debug=False,
                   num_devices=N_CORES)
    f32 = mybir.dt.float32
    bf16 = mybir.dt.bfloat16
    sel_d = nc.dram_tensor("sel", [R_PER_CORE, N_ITEMS], f32,
                           kind="ExternalInput").ap()
    items_d = nc.dram_tensor("items", [N_ITEMS, N_COEFFS],
                             f32, kind="ExternalInput").ap()
    wm_d = nc.dram_tensor("wm", list(wm_np.shape), bf16,
                          kind="ExternalInput").ap()
    ampsel_d = nc.dram_tensor("ampsel", list(ampsel_np.shape),
                              mybir.dt.float32r, kind="ExternalInput").ap()
    ident_d = nc.dram_tensor("ident", [128, 128], f32,
                             kind="ExternalInput").ap()
    negi_d = nc.dram_tensor("negi", [128, 128], mybir.dt.float16,
                            kind="ExternalInput").ap()
    out_d = nc.dram_tensor("out", [R_PER_CORE, NFR * STEP], f32,
                           kind="ExternalOutput").ap()

    with tile.TileContext(nc) as tc:
        with ExitStack() as ctx:
            _kernel_body(ctx, tc, out_d, sel_d, items_d, wm_d,
                         ampsel_d, ident_d, negi_d)
    nc.compile()

    _BUILT = (nc, wm_np, ampsel_np, ident_np, negi_np, corr_np, w256_np)
    return _BUILT


def _in_maps(selections, items, wm_np, ampsel_np, ident_np, negi_np):
    sel_flat = np.ascontiguousarray(
        np.asarray(selections).reshape(NROWS, N_ITEMS).astype(np.float32))
    items_f = np.ascontiguousarray(np.asarray(items).astype(np.float32))
    maps = []
    for c in range(N_CORES):
        maps.append({
            "sel": sel_flat[c * R_PER_CORE:(c + 1) * R_PER_CORE],
            "items": items_f,
            "wm": wm_np,
            "ampsel": ampsel_np,
            "ident": ident_np,
            "negi": negi_np,
        })
    return maps


def _bin256_host(selections, items, w256):
    # bin 256 (Nyquist) spectral line, computed exactly on host: per row a
    # rank-1 [frames x window] contribution folded with hann + overlap-add
    sel = np.maximum(np.asarray(selections, np.float64).reshape(NROWS, N_ITEMS),
                     0.0)
    it = np.asarray(items, np.float64)[:, [256, 256 + CBINS, 256 + 2 * CBINS]]
    c3 = sel @ it                                            # (512, 3)
    mag = 0.5 + 1.0 / (1.0 + np.exp(-c3[:, 0])) * 0.49995
    ph = np.tanh(c3[:, 1]) * np.pi
    st = 1.0 / (1.0 + np.exp(-c3[:, 2]))
    t = np.arange(NT, dtype=np.float64)                      # 0..128
    spec = (st[:, None] * np.exp(np.log(mag + 1e-12)[:, None] * t)
            * np.cos(ph[:, None] * t))                       # (512, 129)
    sw1 = spec[:, 1:NT]                                      # t = f+1
    sw2 = spec[:, 0:NFR].copy()
    sw2[:, 0] = 0.0                                          # no frame -1
    return (sw1[:, :, None] * w256[None, None, :STEP]
            + sw2[:, :, None] * w256[None, None, STEP:])     # (512,128,256)


def kernel(selections: np.ndarray, items: np.ndarray) -> np.ndarray:
    from concourse.bass_utils import run_bass_kernel_spmd

    nc, wm_np, ampsel_np, ident_np, negi_np, corr_np, w256_np = _build()
    in_maps = _in_maps(selections, items, wm_np, ampsel_np, ident_np, negi_np)
    res = run_bass_kernel_spmd(nc, in_maps, core_ids=list(range(N_CORES)))
    rows = np.concatenate([res.results[c]["out"] for c in range(N_CORES)], 0)
    rows = rows.astype(np.float64)
    # t=0 slot: theta-const rides sel_1 (zeroed at t0), so cos(t0)=sin(0)=0
    # and RE(t0)=0 on device already -- no correction needed
    rows += _bin256_host(selections, items, w256_np).reshape(NROWS, -1)
    norms = np.linalg.norm(rows, axis=-1, keepdims=True)
    rows = rows / (norms + 1e-8)
    sh = np.asarray(selections).shape
    return rows.reshape(sh[0], sh[1], sh[2], NFR * STEP).astype(np.float32)


# revision 99
# speedup vs baseline: 1.0875x; 1.0136x over previous
"""Trainium2 Bass kernel for nn_FFTResonanceLookup.

Full inputs: selections (8,16,4,512) f32, items (512,771) f32.
Output: (8,16,4,32768) f32, unit-normalized along samples.

Data-parallel over the 512 (b,e,x) rows -> 64 rows/core x 8 cores.
Per row the synth is a 1024-feature x 256-sample matmul (irfft + hann
window + 50%-overlap-add folded into constant weights). Two phases so
the Act engine never thrashes activation tables: (1) all arg matmuls +
Exp (Ln/Exp share a table), (2) theta matmul pairs with magic-number
round + negi-matmul subtract (reduced phase stays f32 in PSUM), one
Sin per unit, spec products, and per-row DFT matmuls interleaved as
spec pieces complete. Bin 256 (Nyquist) is a rank-1 per-row term added
on host in f64; normalization is also on host.
"""

import math
from contextlib import ExitStack

import numpy as np

N_ITEMS = 512
N_COEFFS = 771
CBINS = 257
WIN = 512
STEP = 256
NFR = 128
NT = NFR + 1            # t = 0..128
NT_S = 136              # padded per-row stride: 64*136 = 8704 = 17*512
R_PER_CORE = 64
N_CORES = 8
NROWS = 512
PIECE = 512
NPIECE = 17
FREE = R_PER_CORE * NT_S  # 8704

# piece p -> rows whose DFT windows are fully covered once piece p done
ROWS_AT_PIECE = [[] for _ in range(NPIECE)]
for _r in range(R_PER_CORE):
    ROWS_AT_PIECE[(_r * NT_S + NFR) // PIECE].append(_r)

ASL_CHUNKS = 8          # ampsel DMA split for early pipeline start
# staging-copy engine cycle: g=gpsimd(Pool), v=vector(DVE), a=scalar(Act)
COPY_CYCLE = "a"
OUT_GROUP = 4           # rows per output DMA

_BUILT = None


def _build_consts():
    import ml_dtypes
    hann = np.hanning(WIN)
    k = np.arange(CBINS)[:, None]
    n = np.arange(WIN)[None, :]
    ang = 2.0 * np.pi * k * n / WIN
    Cm = np.cos(ang) / WIN * np.where((k >= 1) & (k <= 255), 2.0, 1.0)
    Sm = -np.sin(ang) / WIN * np.where((k >= 1) & (k <= 255), 2.0, 0.0)
    Cw = Cm * hann[None, :]
    Sw = Sm * hann[None, :]
    W1 = np.concatenate([Cw[:256, :STEP], Sw[:256, :STEP]], 0)   # (512,256)
    W2 = np.concatenate([Cw[:256, STEP:], Sw[:256, STEP:]], 0)   # (512,256)
    wm = np.zeros((128, 8 * 256))
    for j in range(4):
        wm[:, 256 * j:256 * (j + 1)] = W1[128 * j:128 * (j + 1), :]
        wm[:, 256 * (j + 4):256 * (j + 5)] = W2[128 * j:128 * (j + 1), :]
    # bin 256 is added on the host (rank-1 per row); the im-bin-0 rows of
    # the Sw chunks stay zero, so IM[0] partition 0 can hold anything.
    wm_bf = wm.astype(ml_dtypes.bfloat16)
    w256 = Cw[256, :]                                # (512,) f64 basis

    # every row's t=0 slot produces RE = -1 (amp=1, cos arg = -pi) across all
    # 256 bins; its W2 contribution (chunks 4,5) is a constant per-sample
    # vector cancelled on host after the device run.
    wm_f64 = wm_bf.astype(np.float64)
    corr = (wm_f64[:, 4 * 256:5 * 256].sum(0)
            + wm_f64[:, 5 * 256:6 * 256].sum(0)).astype(np.float32)

    t = np.arange(NT, dtype=np.float64)
    sel_t = np.zeros((64, FREE))
    sel_1 = np.zeros((64, FREE))
    for r in range(64):
        sel_t[r, r * NT_S:r * NT_S + NT] = t
        sel_1[r, r * NT_S + 1:r * NT_S + NT] = 1.0   # t=0 slot stays 0
    ampsel = np.concatenate([sel_t, sel_1], 0)       # (128,8704)
    return (wm_bf, ampsel.astype(np.float32), corr, w256)


def _kernel_body(ctx: ExitStack, tc, out_d, sel_d, items_d, wm_d,
                 ampsel_d, ident_d, negi_d):
    import concourse.mybir as mybir

    nc = tc.nc
    f32 = mybir.dt.float32
    f32r = mybir.dt.float32r
    f16 = mybir.dt.float16
    bf16 = mybir.dt.bfloat16
    AF = mybir.ActivationFunctionType
    OP = mybir.AluOpType
    PI = math.pi
    MAGIC = 12582912.0

    const = ctx.enter_context(tc.tile_pool(name="const", bufs=1))
    persist = ctx.enter_context(tc.tile_pool(name="persist", bufs=1))

    _pn = [0]

    def p2tile(shape, name):
        _pn[0] += 1
        return pst2.tile(shape, f32, tag="p2", name=f"{name}{_pn[0]}")

    # scoped pools: ampsel (dead after phase 1) and stage-1/2 transients
    # (dead after stage 2); released zones are reused by phase-2 pools.
    # Created in reverse release order (pool stack is LIFO).
    ppam = tc.tile_pool(name="ppam", bufs=2, space="PSUM")
    ppa = ppam.__enter__()
    pst2p = tc.tile_pool(name="pst2", bufs=2, space="PSUM")
    pst2 = pst2p.__enter__()
    s2p = tc.tile_pool(name="s2p", bufs=1)
    s2 = s2p.__enter__()

    # ---- constants + inputs (SP DMA queue is in-order: stage-2 inputs
    # first, then ampsel chunks, wm last — first needed only at first DFT)
    it4 = s2.tile([128, 4 * N_COEFFS + 4], f32)
    items_v = items_d.rearrange("(a p) c -> p a c", p=128)
    nc.sync.dma_start(it4[:, 0:4 * N_COEFFS].rearrange("p (a c) -> p a c", a=4),
                      items_v)
    zpad = s2.tile([128, 4], f32)
    nc.vector.memset(zpad[:], 0.0)
    nc.vector.tensor_copy(it4[:, 4 * N_COEFFS:], zpad[:])
    sel_t = s2.tile([64, 512], f32)
    nc.sync.dma_start(sel_t[:], sel_d[:])
    ident = const.tile([128, 128], f32)
    nc.sync.dma_start(ident[:], ident_d[:])
    asl = const.tile([128, FREE], f32r)
    ck = FREE // ASL_CHUNKS
    for i in range(ASL_CHUNKS):
        nc.sync.dma_start(asl[:, i * ck:(i + 1) * ck],
                          ampsel_d[:, i * ck:(i + 1) * ck])
    wm = const.tile([128, 2048], bf16)
    nc.sync.dma_start(wm[:], wm_d[:])
    bias_half = const.tile([128, 1], f32)
    nc.vector.memset(bias_half[:], 0.5)
    bias_npi = const.tile([128, 1], f32)
    nc.vector.memset(bias_npi[:], -PI)
    zeros64 = const.tile([64, 128], f32)
    nc.vector.memset(zeros64[:], 0.0)
    quart64 = const.tile([64, 128], f32)
    nc.vector.memset(quart64[:], 0.25)
    negi = const.tile([128, 128], f16)
    nc.sync.dma_start(negi[:], negi_d[:])

    rs = s2.tile([64, 512], f32)
    nc.scalar.activation(rs[:], sel_t[:], AF.Relu)

    rsT = []
    for kc in range(4):
        pt_ = p2tile([128, 64], "tr")
        nc.tensor.transpose(pt_[:], rs[:, kc * 128:(kc + 1) * 128],
                            ident[0:64, 0:64])
        st = s2.tile([128, 64], f32, name=f"rsT{kc}")
        nc.vector.tensor_copy(st[:], pt_[:])
        rsT.append(st)

    # coefA in three 260-wide column groups: mags and starts first (they
    # gate ampx -> the whole exp phase); phases computed after the
    # sigmoids, overlapped, with tanh emitted only after its producer
    coefA = s2.tile([64, N_COEFFS], f32)
    pAm = p2tile([64, 260], "pA")
    for kc in range(4):
        nc.tensor.matmul(pAm[:], rsT[kc][:],
                         it4[:, kc * N_COEFFS:kc * N_COEFFS + 260],
                         start=(kc == 0), stop=(kc == 3))
    pAs = p2tile([64, 260], "pA")
    for kc in range(4):
        nc.tensor.matmul(pAs[:], rsT[kc][:],
                         it4[:, kc * N_COEFFS + 514:kc * N_COEFFS + 774],
                         start=(kc == 0), stop=(kc == 3))
    nc.vector.tensor_copy(coefA[:, 0:257], pAm[:, 0:257])
    nc.vector.tensor_copy(coefA[:, 514:771], pAs[:, 0:257])

    # ---- stage 2: activation blocks ----
    # block A: sigmoid table (sigmoid + tanh)
    sig_mag, sig_st, th = {}, {}, {}
    for c in range(2):
        sm = s2.tile([64, 128], f32, name=f"sigmag{c}")
        nc.scalar.activation(sm[:], coefA[:, c * 128:(c + 1) * 128], AF.Sigmoid)
        sig_mag[c] = sm
        ss = s2.tile([64, 128], f32, name=f"sigst{c}")
        nc.scalar.activation(ss[:], coefA[:, 2 * CBINS + c * 128:2 * CBINS + (c + 1) * 128],
                             AF.Sigmoid)
        sig_st[c] = ss

    # deferred phases columns; tanh emitted AFTER its producer so the
    # scheduler cannot order it ahead of the copy (still sigmoid table)
    pAp = p2tile([64, 260], "pA")
    for kc in range(4):
        nc.tensor.matmul(pAp[:], rsT[kc][:],
                         it4[:, kc * N_COEFFS + 257:kc * N_COEFFS + 517],
                         start=(kc == 0), stop=(kc == 3))
    nc.vector.tensor_copy(coefA[:, 257:514], pAp[:, 0:257])
    for c in range(2):
        tt = s2.tile([64, 128], f32, name=f"th{c}")
        nc.scalar.activation(tt[:], coefA[:, CBINS + c * 128:CBINS + (c + 1) * 128],
                             AF.Tanh)
        th[c] = tt

    # weight tiles for the arg matmuls
    ampx, thT, thTc = {}, {}, {}
    for c in range(2):
        ax = persist.tile([128, 128], f32r, name=f"ampx{c}")
        ampx[c] = ax
        tx = persist.tile([128, 128], f32r, name=f"thT{c}")
        nc.vector.tensor_copy(tx[64:128, :], zeros64[:])
        nc.vector.tensor_scalar(tx[0:64, :], th[c][:], 0.5, None, OP.mult)
        thT[c] = tx
        txc = persist.tile([128, 128], f32r, name=f"thTc{c}")
        nc.vector.tensor_copy(txc[64:128, :], quart64[:])
        nc.vector.tensor_copy(txc[0:64, :], tx[0:64, :])
        thTc[c] = txc

    # block B: natural_log_exp table (Ln now, Exp pieces later share it).
    # The fence keeps the scheduler from interleaving sigmoid/tanh with Ln,
    # which would thrash activation tables.
    tc.no_sync_barrier()
    for c in range(2):
        lgm = s2.tile([64, 128], f32, name=f"lgm{c}")
        nc.scalar.activation(lgm[:], sig_mag[c][:], AF.Ln,
                             bias=bias_half[0:64], scale=0.49995)
        nc.vector.tensor_copy(ampx[c][0:64, :], lgm[:])
        lgs = s2.tile([64, 128], f32, name=f"lgs{c}")
        nc.scalar.activation(lgs[:], sig_st[c][:], AF.Ln)
        nc.vector.tensor_copy(ampx[c][64:128, :], lgs[:])
    s2p.__exit__(None, None, None)
    pst2p.__exit__(None, None, None)

    # ---- stage 3+4: two phases (exp+mods, then sin+DFT) ----
    AMP = [persist.tile([128, FREE], bf16, name=f"AMP{i}") for i in range(2)]

    specs = [None, None, None, None]
    cp_i = [0]
    ost_state = {"tile": None, "po": None}
    pending = []            # completed po pair tiles awaiting staging copy

    def emit_row_mm(r):
        # two rows share one PSUM bank; the pair joins `pending` on the
        # odd row and is staged a couple of pieces later (so the staging
        # copy never heads-of-line-blocks the DVE queue behind PE)
        half = r % 2
        if half == 0:
            _pn[0] += 1
            ost_state["po"] = ppo.tile([128, 512], f32, tag="po",
                                       name=f"po{_pn[0]}")
        po = ost_state["po"]
        pv = po[:, half * 256:(half + 1) * 256]
        for j in range(4):
            nc.tensor.matmul(pv,
                             specs[j][:, r * NT_S + 1:r * NT_S + NT],
                             wm[:, j * 256:(j + 1) * 256],
                             start=(j == 0), stop=False)
        for j in range(4):
            nc.tensor.matmul(pv,
                             specs[j][:, r * NT_S:r * NT_S + NFR],
                             wm[:, (j + 4) * 256:(j + 5) * 256],
                             start=False, stop=(j == 3))
        if half == 1:
            pending.append((r - 1, po))

    def flush_pairs(n=None):
        k = len(pending) if n is None else min(n, len(pending))
        for _ in range(k):
            r0p, po = pending.pop(0)
            slot = r0p % OUT_GROUP
            if slot == 0:
                ost_state["tile"] = ostage.tile(
                    [128, OUT_GROUP * 256], f32, tag="ost",
                    name=f"ost{r0p // OUT_GROUP}")
            ost = ost_state["tile"]
            dst = ost[:, slot * 256:(slot + 2) * 256]
            eng = COPY_CYCLE[cp_i[0] % len(COPY_CYCLE)]
            cp_i[0] += 1
            if eng == "v":
                nc.vector.tensor_copy(dst, po[:])
            else:
                nc.scalar.copy(dst, po[:])
            if slot == OUT_GROUP - 2:
                r0 = r0p - OUT_GROUP + 2
                dstv = out_d[r0:r0 + OUT_GROUP, :].rearrange(
                    "r (f s) -> f r s", f=NFR)
                nc.sync.dma_start(dstv, ost[:].rearrange(
                    "f (r s) -> f r s", r=OUT_GROUP))

    # --- phase 1: arg matmuls + Exp activations (Ln/Exp share a table) ---
    tc.no_sync_barrier()
    for p0 in range(0, NPIECE - 1, 2):
        w0 = p0 * PIECE
        for c in range(2):
            _pn[0] += 1
            pa = ppa.tile([128, 2 * PIECE], f32, tag="pa", name=f"pa{_pn[0]}")
            nc.tensor.matmul(pa[:, 0:PIECE], ampx[c][:],
                             asl[:, w0:w0 + PIECE], start=True, stop=True)
            nc.tensor.matmul(pa[:, PIECE:], ampx[c][:],
                             asl[:, w0 + PIECE:w0 + 2 * PIECE],
                             start=True, stop=True)
            nc.scalar.activation(AMP[c][:, w0:w0 + 2 * PIECE], pa[:], AF.Exp)
    w0 = (NPIECE - 1) * PIECE
    for c in range(2):
        _pn[0] += 1
        pa = ppa.tile([128, PIECE], f32, tag="pa", name=f"pa{_pn[0]}")
        nc.tensor.matmul(pa[:], ampx[c][:], asl[:, w0:w0 + PIECE],
                         start=True, stop=True)
        nc.scalar.activation(AMP[c][:, w0:w0 + PIECE], pa[:], AF.Exp)
    ppam.__exit__(None, None, None)

    # --- phase 2: theta matmul pairs, magic-round + negi subtract (u stays
    # f32 in PSUM), one Sin per unit, spec products, DFT + output.
    pptp = ctx.enter_context(tc.tile_pool(name="pptp", bufs=3, space="PSUM"))
    ppo = ctx.enter_context(tc.tile_pool(name="ppo", bufs=2, space="PSUM"))
    reim = ctx.enter_context(tc.tile_pool(name="reim", bufs=1))
    rspool = ctx.enter_context(tc.tile_pool(name="rspool", bufs=4))
    scpool = ctx.enter_context(tc.tile_pool(name="scpool", bufs=4))
    ostage = ctx.enter_context(tc.tile_pool(name="ostage", bufs=2))
    RE = [reim.tile([128, FREE], bf16, name=f"RE{i}") for i in range(2)]
    IM = [reim.tile([128, FREE], bf16, name=f"IM{i}") for i in range(2)]
    specs[:] = [RE[0], RE[1], IM[0], IM[1]]

    units = [(p, c) for p in range(NPIECE) for c in range(2)]
    ustate = {}

    def emit_ptmm_rs(i):
        p, c = units[i]
        w0, w1 = p * PIECE, (p + 1) * PIECE
        _pn[0] += 1
        ptp = pptp.tile([128, 2 * PIECE], f32, tag="ptp", name=f"ptp{_pn[0]}")
        nc.tensor.matmul(ptp[:, 0:PIECE], thT[c][:], asl[:, w0:w1],
                         start=True, stop=True)
        nc.tensor.matmul(ptp[:, PIECE:], thTc[c][:], asl[:, w0:w1],
                         start=True, stop=True)
        rs_ = rspool.tile([128, 2 * PIECE], f16, tag="rs", name=f"rs{_pn[0]}")
        nc.vector.tensor_scalar(rs_[:], ptp[:], MAGIC, MAGIC,
                                OP.add, OP.subtract)
        ustate[i] = (ptp, rs_)

    def emit_unit_tail(i):
        p, c = units[i]
        w0, w1 = p * PIECE, (p + 1) * PIECE
        ptp, rs_ = ustate.pop(i)
        nc.tensor.matmul(ptp[:, 0:PIECE], negi[:], rs_[:, 0:PIECE],
                         start=False, stop=True, skip_group_check=True)
        nc.tensor.matmul(ptp[:, PIECE:], negi[:], rs_[:, PIECE:],
                         start=False, stop=True, skip_group_check=True)
        _pn[0] += 1
        sc = scpool.tile([128, 2 * PIECE], bf16, tag="sc", name=f"sc{_pn[0]}")
        nc.scalar.activation(sc[:], ptp[:], AF.Sin, scale=2.0 * PI)
        # sc[:, :PIECE] = sin, sc[:, PIECE:] = cos
        nc.vector.tensor_tensor(RE[c][:, w0:w1], AMP[c][:, w0:w1],
                                sc[:, PIECE:], OP.mult)
        nc.vector.tensor_tensor(IM[c][:, w0:w1], AMP[c][:, w0:w1],
                                sc[:, 0:PIECE], OP.mult)

    tc.no_sync_barrier()
    emit_ptmm_rs(0)
    for i in range(len(units)):
        p, c = units[i]
        if i + 1 < len(units):
            emit_ptmm_rs(i + 1)
        emit_unit_tail(i)
        if c == 1:
            for r in ROWS_AT_PIECE[p]:
                emit_row_mm(r)
                flush_pairs()
    flush_pairs()


def _build():
    global _BUILT
    if _BUILT is not None:
        return _BUILT
    import concourse.bacc as bacc
    import concourse.mybir as mybir
    import concourse.tile as tile

    wm_np, ampsel_np, corr_np, w256_np = _build_consts()
    ident_np = np.eye(128, dtype=np.float32)
    negi_np = -np.eye(128, dtype=np.float16)

    nc = bacc.Bacc("TRN2", target_bir_lowering=False, debug=False,
                   num_devices=N_CORES)
    f32 = mybir.dt.float32
    bf16 = mybir.dt.bfloat16
    sel_d = nc.dram_tensor("sel", [R_PER_CORE, N_ITEMS], f32,
                           kind="ExternalInput").ap()
    items_d = nc.dram_tensor("items", [N_ITEMS, N_COEFFS],
                             f32, kind="ExternalInput").ap()
    wm_d = nc.dram_tensor("wm", list(wm_np.shape), bf16,
                          kind="ExternalInput").ap()
    ampsel_d = nc.dram_tensor("ampsel", list(ampsel_np.shape),
                              mybir.dt.float32r, kind="ExternalInput").ap()
    ident_d = nc.dram_tensor("ident", [128, 128], f32,
                             kind="ExternalInput").ap()
    negi_d = nc.dram_tensor("negi", [128, 128], mybir.dt.float16,
                            kind="ExternalInput").ap()
    out_d = nc.dram_tensor("out", [R_PER_CORE, NFR * STEP], f32,
                           kind="ExternalOutput").ap()

    with tile.TileContext(nc) as tc:
        with ExitStack() as ctx:
            _kernel_body(ctx, tc, out_d, sel_d, items_d, wm_d,
                         ampsel_d, ident_d, negi_d)
    nc.compile()

    _BUILT = (nc, wm_np, ampsel_np, ident_np, negi_np, corr_np, w256_np)
    return _BUILT


def _in_maps(selections, items, wm_np, ampsel_np, ident_np, negi_np):
    sel_flat = np.ascontiguousarray(
        np.asarray(selections).reshape(NROWS, N_ITEMS).astype(np.float32))
    items_f = np.ascontiguousarray(np.asarray(items).astype(np.float32))
    maps = []
    for c in range(N_CORES):
        maps.append({
            "sel": sel_flat[c * R_PER_CORE:(c + 1) * R_PER_CORE],
            "items": items_f,
            "wm": wm_np,
            "ampsel": ampsel_np,
            "ident": ident_np,
            "negi": negi_np,
        })
    return maps


def _bin256_host(selections, items, w256):
    # bin 256 (Nyquist) spectral line, computed exactly on host: per row a
    # rank-1 [frames x window] contribution folded with hann + overlap-add
    sel = np.maximum(np.asarray(selections, np.float64).reshape(NROWS, N_ITEMS),
                     0.0)
    it = np.asarray(items, np.float64)[:, [256, 256 + CBINS, 256 + 2 * CBINS]]
    c3 = sel @ it                                            # (512, 3)
    mag = 0.5 + 1.0 / (1.0 + np.exp(-c3[:, 0])) * 0.49995
    ph = np.tanh(c3[:, 1]) * np.pi
    st = 1.0 / (1.0 + np.exp(-c3[:, 2]))
    t = np.arange(NT, dtype=np.float64)                      # 0..128
    spec = (st[:, None] * np.exp(np.log(mag + 1e-12)[:, None] * t)
            * np.cos(ph[:, None] * t))                       # (512, 129)
    sw1 = spec[:, 1:NT]                                      # t = f+1
    sw2 = spec[:, 0:NFR].copy()
    sw2[:, 0] = 0.0                                          # no frame -1
    return (sw1[:, :, None] * w256[None, None, :STEP]
            + sw2[:, :, None] * w256[None, None, STEP:])     # (512,128,256)


def kernel(selections: np.ndarray, items: np.ndarray) -> np.ndarray:
    from concourse.bass_utils import run_bass_kernel_spmd

    nc, wm_np, ampsel_np, ident_np, negi_np, corr_np, w256_np = _build()
    in_maps = _in_maps(selections, items, wm_np, ampsel_np, ident_np, negi_np)
    res = run_bass_kernel_spmd(nc, in_maps, core_ids=list(range(N_CORES)))
    rows = np.concatenate([res.results[c]["out"] for c in range(N_CORES)], 0)
    rows = rows.astype(np.float64)
    # t=0 slot: theta-const rides sel_1 (zeroed at t0), so cos(t0)=sin(0)=0
    # and RE(t0)=0 on device already -- no correction needed
    rows += _bin256_host(selections, items, w256_np).reshape(NROWS, -1)
    norms = np.linalg.norm(rows, axis=-1, keepdims=True)
    rows = rows / (norms + 1e-8)
    sh = np.asarray(selections).shape
    return rows.reshape(sh[0], sh[1], sh[2], NFR * STEP).astype(np.float32)
